# revision 6
# baseline (speedup 1.0000x reference)
"""DLRM dot-interaction kernel for Trainium2 (8 NeuronCores, batch-sharded).

Per sample b: T = concat(dense[b], embs[b]) -> [27, 128]; Z = T @ T^T;
output = strict upper triangle of Z -> [351] fp32.

Per-core plan (2048 samples, 16 blocks of 128), v3:
  - SWDGE cast-DMA loads input blocks as [128 b, (f,d)] fp16.
  - PE transposes each [128 b, 128 d] feature slab into PSUM; copies land
    in b-major Tt [128 d, b*32+f] fp16 (32-feature pitch) so a 4-sample
    weight group is 128 contiguous columns.
  - Packed Gram matmuls: per 4-sample group one LDWEIGHTS (128 contiguous
    cols) + one 108-col matmul -> PSUM [128, 108-of-128]; diagonal 27x27
    blocks are the per-sample Grams.  Software-pipelined: transposes of
    block k interleave with Gram matmuls of block k-1 so the PE's HAM
    clock stays warm.
  - DVE/ACT/GPSIMD copy diag blocks to SBUF Zs [(g,m) part, (t,q,n)] fp32
    per half-core (8 blocks).
  - Triu pack straight to DRAM: per half 26 HWDGE DMAs (one per m, 2 free
    dims: [4 part, 256 rows, 26-m run]) scatter z[m, m+1:27] into
    out[s, off_m:]; no DRAM scratch bounce.
"""

import numpy as np

B, NUM_EMBS, D = 16384, 26, 128
N_CORES = 8
BC = B // N_CORES  # 2048 samples per core
BLK = 128          # samples per block
NF = NUM_EMBS + 1  # 27 features
FP = 32            # feature pitch in Tt (27 + 5 junk pad slots)
NPAIR = NF * (NF - 1) // 2  # 351

_CACHE = {}


def build(bc=BC):
    import concourse.bacc as bacc
    import concourse.mybir as mybir
    from concourse.tile import TileContext
    from concourse.masks import make_identity

    fp16 = mybir.dt.float16
    fp32 = mybir.dt.float32

    nc = bacc.Bacc("TRN2", target_bir_lowering=False, debug=False)
    dense_t = nc.dram_tensor("dense", (bc, D), fp32, kind="ExternalInput")
    embs_t = nc.dram_tensor("embs", (bc, NUM_EMBS, D), fp32, kind="ExternalInput")
    out_t = nc.dram_tensor("out", (bc, NPAIR), fp32, kind="ExternalOutput")

    nblk = bc // BLK
    HBLK = 8             # blocks per half (pack-out granularity)
    nq = BLK // 4        # 32 4-sample groups per block
    ZT = 8               # groups per PSUM Z tile

    groups = []
    b = 0
    head = [1, 1, 2]
    while b < nblk:
        sz = min(head.pop(0) if head else 4, nblk - b)
        groups.append((b, sz))
        b += sz
    g_of = {}
    for gs, sz in groups:
        for i in range(sz):
            g_of[gs + i] = (gs, sz)

    off = [0] * NF
    for m in range(1, NF):
        off[m] = off[m - 1] + (NF - m)

    chunks = [7, 7, 7, 6]  # feature chunks per transpose phase

    with TileContext(nc) as tc:
        with (
            tc.tile_pool(name="consts", bufs=1) as consts,
            tc.tile_pool(name="xin", bufs=2) as xpool,
            tc.tile_pool(name="tt", bufs=4) as ttpool,
            tc.tile_pool(name="zs", bufs=2) as zspool,
            tc.tile_pool(name="tp", bufs=2, space="PSUM") as tppool,
            tc.tile_pool(name="zp", bufs=3, space="PSUM") as zppool,
        ):
            ident = consts.tile([128, 128], fp16)
            make_identity(nc, ident)

            dview = dense_t.ap()  # [bc, 128]
            eview = embs_t.ap().rearrange("b f d -> b (f d)")  # [bc, 3328]
            oview = out_t.ap()  # [bc, 351]

            X = None
            Zs = None
            tts = {}   # live Tt tiles by block
            cp_i = 0   # round-robin copy-engine counter
            cp_engs = ("v", "s")  # gpsimd cannot access PSUM

            def do_copy(dst, src):
                nonlocal cp_i
                e = cp_engs[cp_i % len(cp_engs)]
                if e == "v":
                    nc.vector.tensor_copy(out=dst, in_=src)
                elif e == "s":
                    nc.scalar.copy(dst, src)
                else:
                    nc.gpsimd.tensor_copy(out=dst, in_=src)
                cp_i += 1

            def pack_out(h, Zs_h):
                # 26 HWDGE DMAs: triu-pack straight to DRAM out rows
                Zr = Zs_h.rearrange("(gg m) (tq n) -> gg m tq n", gg=4, n=NF)
                ovh = oview[h * HBLK * BLK : (h + 1) * HBLK * BLK].rearrange(
                    "(tq gg) p -> gg tq p", gg=4
                )  # [4, 256, 351]
                for m in range(NF - 1):
                    ln = NF - 1 - m
                    src = Zr[:, m, :, m + 1 : NF]          # [4, 256, ln]
                    dst = ovh[:, :, off[m] : off[m] + ln]  # [4, 256, ln]
                    nc.gpsimd.dma_start(out=dst, in_=src)

            for blk in range(nblk + 1):
                if blk < nblk and blk % HBLK == 0:
                    Zs_new = zspool.tile([128, HBLK * nq * NF], fp32, tag="Zs")
                if blk < nblk:
                    gs, gsz = g_of[blk]
                    if blk == gs:
                        X = xpool.tile([BLK, gsz * NF * D], fp16, tag="X")
                        dsrc = dview[gs * BLK : (gs + gsz) * BLK].rearrange(
                            "(t b) d -> b t d", t=gsz
                        )
                        xd = X.rearrange("b (t c) -> b t c", t=gsz)
                        nc.gpsimd.dma_start(out=xd[:, :, 0:D], in_=dsrc)
                        esrc = eview[gs * BLK : (gs + gsz) * BLK].rearrange(
                            "(t b) c -> b t c", t=gsz
                        )
                        nc.gpsimd.dma_start(out=xd[:, :, D:], in_=esrc)
                    xoff = (blk - gs) * NF * D
                    Tt = ttpool.tile([128, BLK * FP], fp16, tag="Tt")
                    Ttb = Tt.rearrange("d (b f) -> d b f", f=FP)
                    tts[blk] = (Tt, Ttb)

                # interleave: transpose chunk p of blk with Gram zt=p of blk-1
                c0 = 0
                for phase in range(4):
                    cf = chunks[phase]
                    if blk < nblk:
                        tp = tppool.tile([128, 8 * BLK], fp16, tag="tp")
                        for j in range(cf):
                            f = c0 + j
                            nc.tensor.transpose(
                                tp[:, j * BLK : (j + 1) * BLK],
                                X[:, xoff + f * D : xoff + (f + 1) * D],
                                ident,
                            )
                        dst = Ttb[:, :, c0 : c0 + cf]
                        src = tp.rearrange("d (j b) -> d b j", b=BLK)[:, :, :cf]
                        do_copy(dst, src)
                    if blk > 0:
                        kk = blk - 1
                        TtK, TtbK = tts[kk]
                        zp = zppool.tile([128, ZT * 128], fp32, tag="zp")
                        zpc = zp.rearrange("p (q c) -> p q c", q=ZT)
                        for q in range(ZT):
                            qg = phase * ZT + q
                            wop = TtK[:, 4 * qg * FP : (4 * qg + 4) * FP]
                            mop = TtbK[:, 4 * qg : 4 * qg + 4, :NF]
                            nc.tensor.matmul(
                                zpc[:, q, : 4 * NF], wop, mop,
                                start=True, stop=True,
                            )
                        Zsr = Zs.rearrange(
                            "p (t q n) -> p t q n", t=HBLK, q=nq
                        )
                        for g in range(4):
                            srcz = zpc[
                                32 * g : 32 * g + NF, :, NF * g : NF * (g + 1)
                            ]
                            dstz = Zsr[
                                32 * g : 32 * g + NF,
                                kk % HBLK,
                                phase * ZT : (phase + 1) * ZT,
                                :,
                            ]
                            do_copy(dstz, srcz)
                    c0 += cf

                if blk > 0 and (blk - 1) % HBLK == HBLK - 1:
                    pack_out((blk - 1) // HBLK, Zs)
                    tts.pop(blk - 2, None)
                if blk < nblk and blk % HBLK == 0:
                    Zs = Zs_new
                # free old Tt refs (pool rotation handles actual reuse)
                tts.pop(blk - 2, None)

    nc.compile()
    return nc


def _get(bc=BC):
    if bc not in _CACHE:
        _CACHE[bc] = build(bc)
    return _CACHE[bc]


def kernel(dense: np.ndarray, embs: np.ndarray) -> np.ndarray:
    from concourse import bass_utils

    dense = np.ascontiguousarray(np.asarray(dense, dtype=np.float32))
    embs = np.ascontiguousarray(np.asarray(embs, dtype=np.float32))
    assert dense.shape == (B, D) and embs.shape == (B, NUM_EMBS, D)

    nc = _get()
    dsh = dense.reshape(N_CORES, BC, D)
    esh = embs.reshape(N_CORES, BC, NUM_EMBS, D)
    in_maps = [{"dense": dsh[i], "embs": esh[i]} for i in range(N_CORES)]
    res = bass_utils.run_bass_kernel_spmd(nc, in_maps, core_ids=list(range(N_CORES)))
    return np.concatenate([r["out"] for r in res.results], axis=0)


# revision 10
# speedup vs baseline: 1.6849x; 1.6849x over previous
"""DLRM dot-interaction kernel for Trainium2 (8 NeuronCores, batch-sharded).

Per sample b: T = concat(dense[b], embs[b]) -> [27, 128]; Z = T @ T^T;
output = strict upper triangle of Z -> [351] fp32.

Per-core plan (2048 samples, 16 blocks of 128), v5:
  - SWDGE cast-DMA loads input blocks as [128 b, (f,d)] fp16.
  - PE transposes feature slabs into PSUM; copies land in b-major
    Tt [128 d, b*32+f] fp16 so a 4-sample weight group is 128 contiguous
    columns (walrus needs a single free dim on the stationary operand).
  - Packed Gram matmuls: per 4-sample group one LDWEIGHTS (128 cols) +
    one 108-col matmul; diagonal 27x27 blocks are the per-sample Grams.
    Transposes of block k interleave with Grams of block k-1 (HAM warm).
  - DVE/ACT diag-copies (fp32->fp16) -> Zs [(g,m) part, (q,n)] per block;
    SWDGE bounces Zs to DRAM scratch (contiguous rows, cheap descriptors).
  - Per quarter: HWDGE reload scatters scratch into sample-major
    Zb [(g,q) part, (t,m,n)]; DVE/ACT pack triu into Pk [s, (t,351)] fp32;
    4 contiguous-row DMAs write DRAM out.
"""

import numpy as np

B, NUM_EMBS, D = 16384, 26, 128
N_CORES = 8
BC = B // N_CORES  # 2048 samples per core
BLK = 128          # samples per block
NF = NUM_EMBS + 1  # 27 features
FP = 32            # feature pitch in Tt (27 + 5 junk pad slots)
NPAIR = NF * (NF - 1) // 2  # 351

_CACHE = {}


def build(bc=BC):
    import concourse.bacc as bacc
    import concourse.mybir as mybir
    from concourse.tile import TileContext
    from concourse.masks import make_identity

    fp16 = mybir.dt.float16
    fp32 = mybir.dt.float32

    nc = bacc.Bacc("TRN2", target_bir_lowering=False, debug=False)
    dense_t = nc.dram_tensor("dense", (bc, D), fp32, kind="ExternalInput")
    embs_t = nc.dram_tensor("embs", (bc, NUM_EMBS, D), fp32, kind="ExternalInput")
    out_t = nc.dram_tensor("out", (bc, NPAIR), fp32, kind="ExternalOutput")

    nblk = bc // BLK
    QBLK = 4             # blocks per quarter (reload/pack granularity)
    nq = BLK // 4        # 32 4-sample groups per block
    ZT = 8               # groups per PSUM Z tile

    groups = []
    b = 0
    head = [1, 1, 2]
    while b < nblk:
        sz = min(head.pop(0) if head else 4, nblk - b)
        groups.append((b, sz))
        b += sz
    g_of = {}
    for gs, sz in groups:
        for i in range(sz):
            g_of[gs + i] = (gs, sz)

    off = [0] * NF
    for m in range(1, NF):
        off[m] = off[m - 1] + (NF - m)

    chunks = [7, 7, 7, 6]

    with TileContext(nc) as tc:
        with (
            tc.tile_pool(name="consts", bufs=1) as consts,
            tc.tile_pool(name="xin", bufs=2) as xpool,
            tc.tile_pool(name="tt", bufs=4) as ttpool,
            tc.tile_pool(name="zs", bufs=3) as zspool,
            tc.tile_pool(name="zb", bufs=2) as zbpool,
            tc.tile_pool(name="pk", bufs=2) as pkpool,
            tc.tile_pool(name="tp", bufs=2, space="PSUM") as tppool,
            tc.tile_pool(name="zp", bufs=3, space="PSUM") as zppool,
            tc.tile_pool(name="dscr", bufs=8, space="DRAM") as dpool,
        ):
            ident = consts.tile([128, 128], fp16)
            make_identity(nc, ident)

            dview = dense_t.ap()
            eview = embs_t.ap().rearrange("b f d -> b (f d)")
            oview = out_t.ap()

            X = None
            tts = {}
            zss = {}
            scrs = {}
            cp_i = 0

            def do_copy(dst, src):
                nonlocal cp_i
                if cp_i % 2 == 0:
                    nc.vector.tensor_copy(out=dst, in_=src)
                else:
                    nc.scalar.copy(dst, src)
                cp_i += 1

            def do_quarter(qtr):
                # reload scratch -> sample-major Zb [(g,q) part, (t,m,n)]
                Zb = zbpool.tile([128, QBLK * NF * NF], fp16, tag="Zb")
                zb5 = Zb.rearrange(
                    "(g q) (t m n) -> g q t m n", g=4, t=QBLK, n=NF
                )
                for t in range(QBLK):
                    scr_t = scrs.pop(qtr * QBLK + t)
                    sct = scr_t.rearrange(
                        "(g m) (q n) -> g q m n", g=4, n=NF
                    )  # m = 32 (incl 5 junk rows per g, skipped below)
                    for g in range(4):
                        nc.sync.dma_start(out=zb5[g, :, t], in_=sct[g][:, :NF, :])
                # pack triu -> Pk fp32 (cast in copy)
                Pk = pkpool.tile([128, QBLK * NPAIR], fp32, tag="Pk")
                zbp = Zb.rearrange("p (t c) -> p t c", t=QBLK)
                pkp = Pk.rearrange("p (t c) -> p t c", t=QBLK)
                for m in range(NF - 1):
                    ln = NF - 1 - m
                    src = zbp[:, :, m * NF + m + 1 : m * NF + NF]
                    dst = pkp[:, :, off[m] : off[m] + ln]
                    do_copy(dst, src)
                # final out: contiguous 1404B rows
                ovq = oview[qtr * QBLK * BLK : (qtr + 1) * QBLK * BLK].rearrange(
                    "(t q g) p -> g q t p", g=4, t=QBLK
                )
                pk4 = Pk.rearrange("(g q) (t c) -> g q t c", g=4, t=QBLK)
                for g in range(4):
                    eng = nc.sync if g % 2 == 0 else nc.scalar
                    eng.dma_start(out=ovq[g], in_=pk4[g])

            for blk in range(nblk + 1):
                if blk < nblk:
                    gs, gsz = g_of[blk]
                    if blk == gs:
                        X = xpool.tile([BLK, gsz * NF * D], fp16, tag="X")
                        dsrc = dview[gs * BLK : (gs + gsz) * BLK].rearrange(
                            "(t b) d -> b t d", t=gsz
                        )
                        xd = X.rearrange("b (t c) -> b t c", t=gsz)
                        nc.gpsimd.dma_start(out=xd[:, :, 0:D], in_=dsrc)
                        esrc = eview[gs * BLK : (gs + gsz) * BLK].rearrange(
                            "(t b) c -> b t c", t=gsz
                        )
                        nc.gpsimd.dma_start(out=xd[:, :, D:], in_=esrc)
                    xoff = (blk - gs) * NF * D
                    Tt = ttpool.tile([128, BLK * FP], fp16, tag="Tt")
                    tts[blk] = Tt
                    Zs_b = zspool.tile(
                        [128, nq * NF], fp16, tag="Zs", name=f"Zs_{blk}"
                    )
                    zss[blk] = Zs_b

                c0 = 0
                for phase in range(4):
                    cf = chunks[phase]
                    if blk < nblk:
                        Ttb = tts[blk].rearrange("d (b f) -> d b f", f=FP)
                        tp = tppool.tile([128, 8 * BLK], fp16, tag="tp")
                        for j in range(cf):
                            f = c0 + j
                            nc.tensor.transpose(
                                tp[:, j * BLK : (j + 1) * BLK],
                                X[:, xoff + f * D : xoff + (f + 1) * D],
                                ident,
                            )
                        dst = Ttb[:, :, c0 : c0 + cf]
                        src = tp.rearrange("d (j b) -> d b j", b=BLK)[:, :, :cf]
                        do_copy(dst, src)
                    if blk > 0:
                        kk = blk - 1
                        TtK = tts[kk]
                        TtbK = TtK.rearrange("d (b f) -> d b f", f=FP)
                        Zsr = zss[kk].rearrange("p (q n) -> p q n", q=nq)
                        zp = zppool.tile([128, ZT * 128], fp32, tag="zp")
                        zpc = zp.rearrange("p (q c) -> p q c", q=ZT)
                        for q in range(ZT):
                            qg = phase * ZT + q
                            wop = TtK[:, 4 * qg * FP : (4 * qg + 4) * FP]
                            mop = TtbK[:, 4 * qg : 4 * qg + 4, :NF]
                            nc.tensor.matmul(
                                zpc[:, q, : 4 * NF], wop, mop,
                                start=True, stop=True,
                            )
                        for g in range(4):
                            srcz = zpc[
                                32 * g : 32 * g + NF, :, NF * g : NF * (g + 1)
                            ]
                            dstz = Zsr[
                                32 * g : 32 * g + NF,
                                phase * ZT : (phase + 1) * ZT,
                                :,
                            ]
                            do_copy(dstz, srcz)
                    c0 += cf

                if blk > 0:
                    kk = blk - 1
                    # bounce Zs -> DRAM scratch: 128 contiguous 1728B rows
                    # (rows 32g+27..32g+31 are junk, never reloaded)
                    scr_t = dpool.tile([128, nq * NF], fp16, tag="scr")
                    nc.gpsimd.dma_start(out=scr_t[:, :], in_=zss.pop(kk)[:, :])
                    scrs[kk] = scr_t
                    tts.pop(kk - 1, None)
                    if kk % QBLK == QBLK - 1:
                        do_quarter(kk // QBLK)

    nc.compile()
    return nc


def _get(bc=BC):
    if bc not in _CACHE:
        _CACHE[bc] = build(bc)
    return _CACHE[bc]


def kernel(dense: np.ndarray, embs: np.ndarray) -> np.ndarray:
    from concourse import bass_utils

    dense = np.ascontiguousarray(np.asarray(dense, dtype=np.float32))
    embs = np.ascontiguousarray(np.asarray(embs, dtype=np.float32))
    assert dense.shape == (B, D) and embs.shape == (B, NUM_EMBS, D)

    nc = _get()
    dsh = dense.reshape(N_CORES, BC, D)
    esh = embs.reshape(N_CORES, BC, NUM_EMBS, D)
    in_maps = [{"dense": dsh[i], "embs": esh[i]} for i in range(N_CORES)]
    res = bass_utils.run_bass_kernel_spmd(nc, in_maps, core_ids=list(range(N_CORES)))
    return np.concatenate([r["out"] for r in res.results], axis=0)
